# revision 1
# baseline (speedup 1.0000x reference)
"""Trainium2 Bass kernel for nn_Attention_1013612281902.

Reference computation (per batch b, head h):
    Q = emb @ Wq[h].T            [S,C]
    K = emb_all @ Wk[h].T        [S,KV]
    V = emb_all @ Wv[h].T        [S,KV]
    scores = Q.T @ K / sqrt(KV)  [C,KV]
    normed = instance_norm(scores)       (mean/var over the whole [C,KV] plane)
    probs  = softmax(normed, axis=KV)
    context = probs @ V.T        [C,S]
    out = mean_h(context).T @ Wo.T       [S,C]

Algebraic restructuring (S=4096 >> C=512, KV=960):
    G = emb.T @ emb_all                      [C,KV]   (shared across heads)
    scores = (Wq[h] @ G @ Wk[h].T)/sqrt(KV)
    Pv[h]  = probs[h] @ Wv[h]                [C,KV]
    out    = emb_all @ (mean_h Pv[h]).T @ Wo.T

Sharding: 8 cores = (4 batches) x (2 head-pairs). Core 2b+g computes the
partial output for batch b over heads {2g, 2g+1}; the host adds the two
partials per batch (the head-mean and output projection are linear). No
collective is used: a pairwise AllReduce/AllGather costs ~30-40us of fixed
NRT latency on the critical path, more than the duplicated output-phase
matmuls it would save.

All matmuls run in bf16 (operand rounding ~5e-3 max-rel-err, well inside the
2e-2 gate) except the tiny cross-partition stats matmul (f32r). bf16 halves
HBM traffic and LoadStationary cost vs f32r. emb_all.T is precomputed on the
host so the phase-3 contraction over KV needs no PE transposes; the full-S
emb_all.T (8MB) fits in SBUF because the two heads' Wk/Wv share one
streamed buffer instead of being both resident.
"""

import sys

if "/opt/trn_rl_repo" not in sys.path:
    sys.path.insert(0, "/opt/trn_rl_repo")

from contextlib import ExitStack

import numpy as np
import ml_dtypes

import concourse.bacc as bacc
import concourse.mybir as mybir
import concourse.tile as tile
from concourse.bass_utils import run_bass_kernel_spmd
from concourse.masks import make_identity
from concourse.tile_rust import add_dep_helper

B, S, C, KV, H = 4, 4096, 512, 960, 4
EPS = 1e-5
F32 = mybir.dt.float32
F32R = mybir.dt.float32r
BF16 = mybir.dt.bfloat16

ST = S // 128            # 32 s-tiles
CT = C // 128            # 4 c-tiles
KT = (KV + 127) // 128   # 8 k-tiles (last one has 64 real partitions)
KVP = 128 * KT           # KV padded to 1024


def _kp(t):
    return min(128, KV - t * 128)


def _build_program():
    nc = bacc.Bacc("TRN2", target_bir_lowering=False, debug=False, num_devices=8)

    emb_d = nc.dram_tensor("emb", [S, C], BF16, kind="ExternalInput")
    ea_d = nc.dram_tensor("ea", [S, KV], BF16, kind="ExternalInput")
    eat_d = nc.dram_tensor("eat", [KV, S], BF16, kind="ExternalInput")
    wqt_d = nc.dram_tensor("wqt", [2, C, C], BF16, kind="ExternalInput")
    wkt_d = nc.dram_tensor("wkt", [2, KV, KV], BF16, kind="ExternalInput")
    wv_d = nc.dram_tensor("wv", [2, KV, KV], BF16, kind="ExternalInput")
    wot_d = nc.dram_tensor("wot", [C, C], BF16, kind="ExternalInput")
    y_d = nc.dram_tensor("y", [S, C], BF16, kind="ExternalOutput")

    with tile.TileContext(nc) as tc, ExitStack() as ectx:
        ec = ectx.enter_context
        const = ec(tc.tile_pool(name="const", bufs=1))
        gp = ec(tc.tile_pool(name="gp", bufs=1))
        wqp = ec(tc.tile_pool(name="wqp", bufs=1))
        wkp = ec(tc.tile_pool(name="wkp", bufs=1))
        wvp = ec(tc.tile_pool(name="wvp", bufs=1))
        wop = ec(tc.tile_pool(name="wop", bufs=1))
        eatp = ec(tc.tile_pool(name="eatp", bufs=1))
        embp = ec(tc.tile_pool(name="embp", bufs=8))
        eap = ec(tc.tile_pool(name="eap", bufs=8))
        ap_pool = ec(tc.tile_pool(name="ap", bufs=1))   # A tiles (h0/h1 reuse)
        scp = ec(tc.tile_pool(name="scp", bufs=1))      # scoresT bf16
        ep_pool = ec(tc.tile_pool(name="ep", bufs=1))   # exp(probs) bf16
        pbp = ec(tc.tile_pool(name="pbp", bufs=1))      # Pbar bf16 accumulator
        zp = ec(tc.tile_pool(name="zp", bufs=1))        # pbt + Z
        outp = ec(tc.tile_pool(name="outp", bufs=3))
        srp = ec(tc.tile_pool(name="srp", bufs=2))      # [128,512] scratch
        stp = ec(tc.tile_pool(name="stp", bufs=4))      # small stats tiles

        onesf = const.tile([128, 128], F32)
        nc.vector.memset(onesf[:], 1.0)
        onesr = const.tile([128, 128], F32R)
        nc.vector.tensor_copy(out=onesr[:], in_=onesf[:])
        # scores are left unscaled (instance-norm is scale-invariant), so the
        # reference's eps applies to var/KV: use KV*eps against unscaled var.
        eps_t = const.tile([128, 1], F32)
        nc.vector.memset(eps_t[:], EPS * KV)
        # Scratch for ACT-table prewarming (Sqrt/Exp table loads are ~1.3us;
        # issuing a dummy op early moves the load off the critical chain).
        warm = const.tile([128, 1], F32)
        nc.vector.memset(warm[:], 1.0)

        def prewarm(func, nm):
            wsink = stp.tile([128, 1], F32, tag="wsink", name=nm)
            nc.scalar.activation(out=wsink[:], in_=warm[:], func=func)

        # ---- phase 1: G = emb.T @ emb_all  [C, KV] --------------------------
        g_sb = gp.tile([128, CT, KV], BF16)
        gps_pool = tc.tile_pool(name="gps", bufs=8, space="PSUM")
        ps = gps_pool.__enter__()
        g_ps = [ps.tile([128, 480], F32, tag="ps", name=f"g_ps{i}") for i in range(8)]
        for i in range(30):
            nc.tensor.matmul(
                g_ps[0][:16, 0:16],
                onesr[:, 0:16],
                onesr[:, 0:16],
                start=True,
                stop=True,
            )
        et_dmas = []
        for st in range(ST):
            et = embp.tile([128, C], BF16, tag="emb", name=f"et{st}")
            et_dmas.append(
                nc.sync.dma_start(
                    out=et[:], in_=emb_d.ap()[st * 128 : (st + 1) * 128, :]
                )
            )
            at = eap.tile([128, KV], BF16, tag="ea", name=f"at{st}")
            nc.sync.dma_start(out=at[:], in_=ea_d.ap()[st * 128 : (st + 1) * 128, :])
            for ct in range(CT):
                for kc in range(2):
                    nc.tensor.matmul(
                        g_ps[ct * 2 + kc][:],
                        et[:, ct * 128 : (ct + 1) * 128],
                        at[:, kc * 480 : (kc + 1) * 480],
                        start=(st == 0),
                        stop=(st == ST - 1),
                    )
        for ct in range(CT):
            for kc in range(2):
                # Alternate ACT/DVE so the copy-out tail after the last G
                # matmul drains in half the time.
                dst = g_sb[:, ct, kc * 480 : (kc + 1) * 480]
                if (ct * 2 + kc) % 2 == 0:
                    nc.vector.tensor_copy(out=dst, in_=g_ps[ct * 2 + kc][:])
                else:
                    nc.scalar.copy(out=dst, in_=g_ps[ct * 2 + kc][:])
        gps_pool.__exit__(None, None, None)

        # ---- weights (host provides pre-transposed Wq.T / Wk.T / Wo.T) ------
        # Issued after the G-phase streams so the emb/emb_all DMAs (which
        # gate the first matmuls) get the HBM bandwidth first; within the
        # weights, in consumption order (wqt0 gates phase 2a).
        # Wk/Wv for the two heads share one streamed buffer each (tag reuse):
        # h1's DMA waits on h0's last read, freeing 3.8MB of SBUF for the
        # full-S emb_all.T. Wq is small enough to keep both heads resident.
        def pace(dma, gate):
            if gate is not None:
                add_dep_helper(dma.ins, gate.ins, sync=True, reason="dma pacing")

        wqt_sb = []
        wkt_sb = []
        wv_sb = []
        gates = {0: (et_dmas[16], et_dmas[24]), 1: (et_dmas[31], et_dmas[31])}
        for h in range(2):
            wq_t = wqp.tile([128, CT, C], BF16, tag="wq", name=f"wq{h}")
            pace(
                nc.sync.dma_start(
                    out=wq_t[:],
                    in_=wqt_d.ap()[h].rearrange("(t p) d -> p t d", p=128),
                ),
                gates[h][0] if h == 1 else None,
            )
            wqt_sb.append(wq_t)
            wk_t = wkp.tile([128, KT, KV], BF16, tag="wk", name=f"wk{h}")
            # Wv gets a padded KV=1024 layout: column KV holds 4.0 so the
            # Pv matmuls accumulate 4*sum_j(e) in the pad — the softmax
            # denominator and the 0.25 head-mean factor in one reciprocal.
            wv_t = wvp.tile([128, KT, KVP], BF16, tag="wv", name=f"wv{h}")
            for kt in range(KT):
                kp = _kp(kt)
                pace(
                    nc.sync.dma_start(
                        out=wk_t[:kp, kt, :],
                        in_=wkt_d.ap()[h, kt * 128 : kt * 128 + kp, :],
                    ),
                    gates[h][0],
                )
            for kt in range(KT):
                kp = _kp(kt)
                pace(
                    nc.sync.dma_start(
                        out=wv_t[:kp, kt, 0:KV],
                        in_=wv_d.ap()[h, kt * 128 : kt * 128 + kp, :],
                    ),
                    gates[h][1],
                )
            nc.vector.memset(wv_t[:, :, KV : KV + 1], 4.0)
            nc.vector.memset(wv_t[:, :, KV + 1 :], 0.0)
            wkt_sb.append(wk_t)
            wv_sb.append(wv_t)
        wot_sb = wop.tile([128, CT, C], BF16)
        pace(
            nc.sync.dma_start(
                out=wot_sb[:], in_=wot_d.ap().rearrange("(t p) d -> p t d", p=128)
            ),
            et_dmas[31],
        )
        # Full-S emb_all.T for phase 3 (host-transposed; no PE transposes).
        eat_sb = eatp.tile([128, KT, S], BF16)
        for kt in range(KT):
            kp = _kp(kt)
            pace(
                nc.sync.dma_start(
                    out=eat_sb[:kp, kt, :],
                    in_=eat_d.ap()[kt * 128 : kt * 128 + kp, :],
                ),
                et_dmas[31],
            )
        nc.vector.memset(eat_sb[64:128, KT - 1, :], 0.0)

        # ---- phase 2: per-head scores -> instancenorm -> softmax -> Pv ------
        # The two heads are interleaved: h1's A matmuls are emitted between
        # h0's scoresT and h0's stats/softmax so the PE has work during the
        # (serial) stats chain. One shared PSUM pool spans phase 2 with tags
        # sized to exactly 8 banks: psa(2) + pw(4, shared by scoresT
        # accumulators and Pv accumulators) + one(2, shared by the two tiny
        # stats tiles and the softmax denominator).
        # Pbar.T is written directly by the transposed-Pv copy-outs
        # ([c, kv] layout) — no PE transposes needed in phase 3. The KV pad
        # columns are zeroed so the Z matmuls can run full-width.
        pbt_sb = pbp.tile([128, CT, KVP], BF16)
        nc.vector.memset(pbt_sb[:, :, KV:], 0.0)
        ph2_pool = tc.tile_pool(name="ph2ps", bufs=1, space="PSUM")
        ps = ph2_pool.__enter__()
        hs = [{}, {}]

        def emit_A(h):
            d = hs[h]
            d["a_sb"] = a_sb = ap_pool.tile(
                [128, KT, C], BF16, tag="a", name=f"a_sb{h}"
            )
            for kt in range(KT):
                kp = _kp(kt)
                pa = ps.tile([128, C], F32, tag="psa", bufs=2, name=f"pa{h}{kt}")
                for ct in range(CT):
                    nc.tensor.matmul(
                        pa[:kp, :],
                        g_sb[:, ct, kt * 128 : kt * 128 + kp],
                        wqt_sb[h][:, ct, :],
                        start=(ct == 0),
                        stop=(ct == CT - 1),
                    )
                nc.vector.tensor_copy(out=a_sb[:kp, kt, :], in_=pa[:kp, :])

        def emit_scoresT(h):
            # scoresT[j, d] = sum_k WkT[k,j] A.T[k,d]; the reference's
            # 1/sqrt(KV) scale cancels through instance-norm (eps adjusted).
            # Per-jt stats partials run inline right behind each group.
            d = hs[h]
            a_sb = d["a_sb"]
            d["sc_sb"] = sc_sb = scp.tile(
                [128, KT, C], F32, tag="sc", name=f"sc_sb{h}"
            )
            d["p_sb"] = p_sb = stp.tile([128, 16], F32, tag="p16", name=f"p_sb{h}")
            nc.vector.memset(p_sb[:], 0.0)
            prev_stop = None
            for jt in range(KT):
                jp = _kp(jt)
                pss = ps.tile([128, C], F32, tag="pw", bufs=4, name=f"pss{h}{jt}")
                for kt in range(KT):
                    kp = _kp(kt)
                    mm = nc.tensor.matmul(
                        pss[:jp, :],
                        wkt_sb[h][:kp, kt, jt * 128 : jt * 128 + jp],
                        a_sb[:kp, kt, :],
                        start=(kt == 0),
                        stop=(kt == KT - 1),
                    )
                    # Keep the PE stream jt-group-major: otherwise the
                    # scheduler interleaves the groups and every stop lands
                    # at the tail, stalling the stats.
                    if kt == 0 and prev_stop is not None:
                        add_dep_helper(
                            mm.ins, prev_stop.ins, sync=False, reason="jt order"
                        )
                    if kt == KT - 1:
                        prev_stop = mm
                # Copy-with-accum: the scores copy also produces the row
                # sums, removing the separate DVE reduce from the stats chain.
                nc.scalar.activation(
                    out=sc_sb[:jp, jt, :],
                    in_=pss[:jp, :],
                    func=mybir.ActivationFunctionType.Copy,
                    accum_out=p_sb[:jp, jt : jt + 1],
                )
                nc.scalar.activation(
                    out=pss[:jp, :],
                    in_=pss[:jp, :],
                    func=mybir.ActivationFunctionType.Square,
                    accum_out=p_sb[:jp, 8 + jt : 9 + jt],
                )

        def emit_softmax_pv(h):
            d = hs[h]
            sc_sb = d["sc_sb"]
            p_sb = d["p_sb"]
            # cross-partition reduce + broadcast of the plane stats (f32r).
            p_r = stp.tile([128, 16], F32R, tag="p16r", name=f"p_r{h}")
            nc.vector.tensor_copy(out=p_r[:], in_=p_sb[:])
            pst = ps.tile([128, 16], F32, tag="one", bufs=2, name=f"pst{h}")
            nc.tensor.matmul(pst[:], onesr[:], p_r[:], start=True, stop=True)
            # softmax(x + c) == softmax(x): the instance-norm mean shift
            # cancels, so only rstd = 1/sqrt(var+eps) is needed. (Scores are
            # variance-normalized, so exp(sc*rstd) stays in a safe range.)
            n_inv = 1.0 / float(C * KV)
            t2 = stp.tile([128, 2], F32, tag="sq2", name=f"sq2{h}")
            nc.vector.reduce_sum(
                out=t2[:],
                in_=pst[:].rearrange("p (a b) -> p a b", a=2),
                axis=mybir.AxisListType.X,
            )
            nc.vector.tensor_scalar(
                out=t2[:], in0=t2[:], scalar1=n_inv, scalar2=None,
                op0=mybir.AluOpType.mult,
            )
            m2 = stp.tile([128, 1], F32, tag="m2", name=f"m2{h}")
            nc.vector.tensor_mul(out=m2[:], in0=t2[:, 0:1], in1=t2[:, 0:1])
            var_t = stp.tile([128, 1], F32, tag="var", name=f"var{h}")
            nc.vector.tensor_sub(out=var_t[:], in0=t2[:, 1:2], in1=m2[:])
            std_t = stp.tile([128, 1], F32, tag="std", name=f"std{h}")
            nc.scalar.activation(
                out=std_t[:],
                in_=var_t[:],
                func=mybir.ActivationFunctionType.Sqrt,
                bias=eps_t[:],
            )
            # Swap the ACT table back to Exp while the DVE finishes the chain.
            prewarm(mybir.ActivationFunctionType.Exp, f"wex{h}")
            rstd_t = stp.tile([128, 1], F32, tag="rstd", name=f"rstd{h}")
            nc.vector.reciprocal(out=rstd_t[:], in_=std_t[:])

            # Transposed Pv: stationary = exp d-chunk (4 loads per jt, each
            # reused across both Wv halves — half the weight loads), moving =
            # Wv rows. Output lands directly in the Pbar.T [c, kv] layout that
            # phase 3 consumes, so no PE transposes are needed. The 4.0
            # column in Wv's pad accumulates 4*sum_j(e) per c-row: one
            # [128,1] reciprocal folds the softmax denominator and the 0.25
            # head mean.
            e_sb = ep_pool.tile([128, KT, C], BF16, tag="e", name=f"e_sb{h}")
            tags = (("pw", 4), ("pw", 4), ("psa", 2), ("one", 2))
            pv_ps = [
                [
                    ps.tile(
                        [128, C], F32, tag=tags[ct][0], bufs=tags[ct][1],
                        name=f"pv{h}_{ct}_{half}",
                    )
                    for half in range(2)
                ]
                for ct in range(CT)
            ]
            for jt in range(KT):
                jp = _kp(jt)
                nc.scalar.activation(
                    out=e_sb[:jp, jt, :],
                    in_=sc_sb[:jp, jt, :],
                    func=mybir.ActivationFunctionType.Exp,
                    scale=rstd_t[:jp],
                )
                for ct in range(CT):
                    for half in range(2):
                        nc.tensor.matmul(
                            pv_ps[ct][half][:],
                            e_sb[:jp, jt, ct * 128 : (ct + 1) * 128],
                            wv_sb[h][:jp, jt, half * 512 : (half + 1) * 512],
                            start=(jt == 0),
                            stop=(jt == KT - 1),
                        )
            r4cs = []
            for ct in range(CT):
                r4c = stp.tile([128, 1], F32, tag="r4c", name=f"r4c{h}{ct}")
                nc.vector.reciprocal(
                    out=r4c[:], in_=pv_ps[ct][1][:, KV - 512 : KV - 511]
                )
                r4cs.append(r4c)
            for half in range(2):
                for ct in (2, 3, 0, 1):
                    win = 512 if half == 0 else KV - 512
                    dst = pbt_sb[:, ct, half * 512 : half * 512 + win]
                    src_ = pv_ps[ct][half][:, 0:win]
                    if h == 0:
                        nc.vector.tensor_scalar(
                            out=dst, in0=src_, scalar1=r4cs[ct][:], scalar2=None,
                            op0=mybir.AluOpType.mult,
                        )
                    else:
                        tmp = srp.tile(
                            [128, C], BF16, tag="sr", name=f"tmp{ct}{half}"
                        )
                        nc.vector.tensor_scalar(
                            out=tmp[:, 0:win], in0=src_, scalar1=r4cs[ct][:],
                            scalar2=None, op0=mybir.AluOpType.mult,
                        )
                        nc.vector.tensor_add(out=dst, in0=dst, in1=tmp[:, 0:win])

        emit_A(0)
        emit_scoresT(0)
        emit_A(1)
        emit_softmax_pv(0)
        emit_scoresT(1)
        emit_softmax_pv(1)

        # ---- phase 3: Z = Pbar.T @ Wo.T (local 2-head partial); y = ea @ Z --
        # Reuses the phase-2 PSUM pool: a pool close would barrier phase 3's
        # first allocation on ALL phase-2 banks draining (~7us of PE idle).
        z_sb = zp.tile([128, KT, C], BF16, tag="z")
        for kt in range(KT):
            pz = ps.tile([128, C], F32, tag="psa", bufs=2, name=f"pz{kt}")
            for ct in range(CT):
                nc.tensor.matmul(
                    pz[:],
                    pbt_sb[:, ct, kt * 128 : (kt + 1) * 128],
                    wot_sb[:, ct, :],
                    start=(ct == 0),
                    stop=(ct == CT - 1),
                )
            nc.scalar.copy(out=z_sb[:, kt, :], in_=pz[:])

        # y partial rows: stationary = eaT chunk (host-transposed), moving = Z.
        for st in range(ST):
            po = ps.tile([128, C], F32, tag="pw", bufs=4, name=f"po{st}")
            for kt in range(KT):
                nc.tensor.matmul(
                    po[:],
                    eat_sb[:, kt, st * 128 : (st + 1) * 128],
                    z_sb[:, kt, :],
                    start=(kt == 0),
                    stop=(kt == KT - 1),
                )
            ot = outp.tile([128, C], BF16, tag="out", name=f"ot{st}")
            if st % 2 == 0:
                nc.scalar.copy(out=ot[:], in_=po[:])
            else:
                nc.vector.tensor_copy(out=ot[:], in_=po[:])
            nc.sync.dma_start(out=y_d.ap()[st * 128 : (st + 1) * 128, :], in_=ot[:])
        ph2_pool.__exit__(None, None, None)

    nc.compile()
    return nc


_NC = None


def _get_nc():
    global _NC
    if _NC is None:
        _NC = _build_program()
    return _NC


def _bf(x):
    return np.ascontiguousarray(
        np.asarray(x, dtype=np.float32).astype(ml_dtypes.bfloat16)
    )


def _in_maps(emb, emb_all, Wq, Wk, Wv, Wo):
    wot = _bf(np.asarray(Wo, dtype=np.float32).T)
    wqt_all = np.asarray(Wq, dtype=np.float32).transpose(0, 2, 1)
    wkt_all = np.asarray(Wk, dtype=np.float32).transpose(0, 2, 1)
    eab = [_bf(emb_all[b]) for b in range(B)]
    eatb = [_bf(np.asarray(emb_all[b], dtype=np.float32).T) for b in range(B)]
    embb = [_bf(emb[b]) for b in range(B)]
    maps = []
    for core in range(8):
        b, g = divmod(core, 2)
        h0 = 2 * g
        maps.append(
            {
                "emb": embb[b],
                "ea": eab[b],
                "eat": eatb[b],
                "wqt": _bf(wqt_all[h0 : h0 + 2]),
                "wkt": _bf(wkt_all[h0 : h0 + 2]),
                "wv": _bf(np.asarray(Wv[h0 : h0 + 2], dtype=np.float32)),
                "wot": wot,
            }
        )
    return maps


def run(emb, emb_all, Wq, Wk, Wv, Wo, trace=False):
    nc = _get_nc()
    res = run_bass_kernel_spmd(
        nc, _in_maps(emb, emb_all, Wq, Wk, Wv, Wo), list(range(8)), trace=trace
    )
    out = np.empty((B, S, C), dtype=np.float32)
    for b in range(B):
        out[b] = res.results[2 * b]["y"].astype(np.float32) + res.results[
            2 * b + 1
        ]["y"].astype(np.float32)
    return out, res


def kernel(emb, emb_all, Wq, Wk, Wv, Wo):
    out, _ = run(emb, emb_all, Wq, Wk, Wv, Wo, trace=False)
    return out

